# revision 38
# baseline (speedup 1.0000x reference)
"""BatchedExpertPool Trainium2 kernel.

Computes, for x:[B,L,D], weights:[B,L,E], w1:[E,D,H], b1:[E,H],
w2:[E,H,D], b2:[E,D]:

    h   = gelu(einsum('bld,edh->bleh', x, w1) + b1)      (exact erf gelu)
    out = einsum('bleh,ehd->bled', h, w2) + b2
    ret = einsum('bled,ble->bld', out, weights)

Strategy: data-parallel over the B*L tokens across 8 NeuronCores, expert
params replicated.  Each core processes 1024 tokens entirely on-chip:

  - All matmul operands are bf16 (host-converted).  bf16 stationaries take
    the FWL LDWEIGHTS path (~119ns, fully hidden under the 216ns N=512
    stream); f32r's ~224ns weight load would bind the pipeline instead.
  - Phase 1 computes g = gelu(w1[e].T @ x^T + b1) in [H, tok] layout
    (lhsT = w1 chunk, moving = x^T), accumulating fp32 in PSUM.
  - Phase 2 is flipped to out[tok, D]: lhsT = g chunk (stationary),
    moving = w2[e] rows.  The per-token expert weight is then a
    per-PARTITION scalar, so the weighted accumulation is a single fused
    DVE op: acc = (psum * w[:,None]) + acc, and the token weights for all
    experts load in one 32KB DMA instead of 8 broadcast DMAs.
  - The b2 term (weights @ b2, a rank-1 correction) is added on the host.

Schedule notes (from trace iteration):
  - Prologue to first instruction is ~7us of framework cost (BSP start
    barrier + engine TENSOR_LOADs); scratch warm-up matmuls cover the HAM
    clock-gate ramp while the first x / w1[0] bytes land.
  - All loads ride the two physical HWDGE rings (sync, scalar) in strict
    criticality order; a third (SWDGE) ring would steal a 1/3 round-robin
    share of the ~358GB/s per-core HBM bandwidth from the critical bytes.
  - Token weights and b1 are packed into a single 64KB constant DMA.
  - The output is stored as bf16 (host upcasts) to halve the tail store,
    and the very last matmul group is split in half (N=256) so the final
    DVE drain + store overlap the last matmuls instead of serializing.

Steady state measured on hardware: median inter-matmul gap 0ns, every
matmul at the 216ns N=512 floor; the kernel is within ~4us of the
architectural minimum (221us streaming + fixed prologue/epilogue).
"""

import ml_dtypes
import numpy as np
from contextlib import ExitStack

import concourse.bass as bass
import concourse.tile as tile
from concourse import bacc, mybir
from concourse.bass_utils import run_bass_kernel_spmd

BF16_NP = ml_dtypes.bfloat16

B, L, D, E, H = 4, 2048, 512, 8, 1024
N_CORES = 8
TOK = (B * L) // N_CORES  # tokens per core
T = 512                   # matmul moving-dim tile (one PSUM bank of fp32)
NT = TOK // T             # token tiles per core (phase 1 moving dim)
TC = TOK // 128           # token chunks (phase 2 output partitions)
DC = D // 128             # D chunks
HC = H // 128             # H chunks
N_WARMUP_MM = 10

F32 = mybir.dt.float32
BF16 = mybir.dt.bfloat16
GELU = mybir.ActivationFunctionType.Gelu
MULT = mybir.AluOpType.mult
ADD = mybir.AluOpType.add

_cache: dict = {}


def _build():
    nc = bacc.Bacc(trn_type="TRN2", target_bir_lowering=False, debug=False)

    xT_d = nc.dram_tensor("xT", [D, TOK], BF16, kind="ExternalInput").ap()
    # cst packs the per-token expert weights [128, tc*E] and b1 [128, hc*E]
    # side by side: one DMA, one completion receipt.
    cst_d = nc.dram_tensor("cst", [128, (TC + HC) * E], F32, kind="ExternalInput").ap()
    w1_d = nc.dram_tensor("w1", [E, D, H], BF16, kind="ExternalInput").ap()
    w2_d = nc.dram_tensor("w2", [E, H, D], BF16, kind="ExternalInput").ap()
    out_d = nc.dram_tensor("out", [TOK, D], BF16, kind="ExternalOutput").ap()

    with tile.TileContext(nc) as tc, ExitStack() as ctx:
        consts = ctx.enter_context(tc.tile_pool(name="consts", bufs=1))
        w1p0 = ctx.enter_context(tc.tile_pool(name="w1p0", bufs=4))
        w1p = ctx.enter_context(tc.tile_pool(name="w1p", bufs=3))
        w2p = ctx.enter_context(tc.tile_pool(name="w2p", bufs=2))
        gp = ctx.enter_context(tc.tile_pool(name="gp", bufs=10))
        outp = ctx.enter_context(tc.tile_pool(name="outp", bufs=4))
        php = ctx.enter_context(tc.tile_pool(name="php", bufs=4, space="PSUM"))
        pop = ctx.enter_context(tc.tile_pool(name="pop", bufs=4, space="PSUM"))

        # PE warm-up on scratch data while the first tiles load: the HAM
        # clock gate needs ~3.4us of sustained activity to lift the PE from
        # 1.2 to 2.4 GHz.  memsets on the vector queue (gpsimd's initial
        # drain would delay them past the first DMA descriptors).
        wscr = consts.tile([128, 128], BF16)
        rscr = consts.tile([128, T], BF16)
        nc.vector.memset(wscr[:], 0.0)
        nc.vector.memset(rscr[:], 0.0)
        pscr = php.tile([128, T], F32, tag="ph")
        for _ in range(N_WARMUP_MM):
            nc.tensor.matmul(
                pscr[:], lhsT=wscr[:], rhs=rscr[:], start=True, stop=True,
            )

        # ALL loads go through the two physical HWDGE rings (sync, scalar) in
        # strict criticality order — a third (SWDGE) ring would steal a 1/3
        # round-robin share of the ~358GB/s per-core HBM bandwidth from the
        # first-expert-critical bytes.  x[dc] heads the sync ring while
        # w1[0][dc] heads the scalar ring: the dc-th matmul group of expert 0
        # needs exactly (x[dc], w1[0][dc]).  The packed constants follow the
        # x chunks (first needed by the gelu of expert 0, ~2us later).
        # The very first (x, w1[0]) pair is halved so the first matmul's
        # inputs clear their ~2us completion receipt ~1us sooner; everything
        # else stays at 256KB granularity (finer splits regress — receipts
        # and trigger occupancy dominate below the DMA efficiency knee).
        x0a = consts.tile([128, T], BF16, name="x0a", tag="x0a")
        nc.sync.dma_start(out=x0a[:], in_=xT_d[0:128, 0:T])
        xsb = [[x0a[:, :], None]]
        for dc in range(1, DC):
            xt = consts.tile([128, TOK], BF16, name=f"x{dc}", tag=f"x{dc}")
            nc.sync.dma_start(out=xt[:], in_=xT_d[dc * 128:(dc + 1) * 128, :])
            xsb.append([xt[:, tt * T:(tt + 1) * T] for tt in range(NT)])
        x0b = consts.tile([128, T], BF16, name="x0b", tag="x0b")
        nc.sync.dma_start(out=x0b[:], in_=xT_d[0:128, T:TOK])
        xsb[0][1] = x0b[:, :]

        cst = consts.tile([128, (TC + HC) * E], F32)
        nc.sync.dma_start(out=cst[:], in_=cst_d)

        def w_pp_ap(tcb, e):
            # per-partition token weight column for (token chunk, expert)
            return cst[:, tcb * E + e:tcb * E + e + 1]

        def b1_ap(hc, e):
            return cst[:, (TC + hc) * E + e:(TC + hc) * E + e + 1]

        # fp32 accumulators, one [128 tok, D] tile per token chunk
        acc = [
            consts.tile([128, D], F32, name=f"acc{t}", tag=f"acc{t}")
            for t in range(TC)
        ]

        for e in range(E):
            # w1[e]: expert 0 per-dc chunks (critical path: each landed chunk
            # unlocks matmuls); later experts in one DMA.
            if e == 0:
                w1d0 = []
                for hh in range(2):
                    t = w1p0.tile([128, H // 2], BF16, tag=f"w1e0d0{hh}")
                    nc.scalar.dma_start(
                        out=t[:],
                        in_=w1_d[e, 0:128, hh * (H // 2):(hh + 1) * (H // 2)],
                    )
                    w1d0.append(t)
                w1t = []
                for dc in range(1, DC):
                    t = w1p0.tile([128, H], BF16, tag="w1e0")
                    nc.scalar.dma_start(
                        out=t[:], in_=w1_d[e, dc * 128:(dc + 1) * 128, :]
                    )
                    w1t.append(t)

                def w1sel(dc, hc, w1t=w1t, w1d0=w1d0):
                    if dc == 0:
                        t = w1d0[hc // 4]
                        h = hc % 4
                        return t[:, h * 128:(h + 1) * 128]
                    return w1t[dc - 1][:, hc * 128:(hc + 1) * 128]
            else:
                # Steady state: w1[e] on the scalar ring, w2[e] on the sync
                # ring, each behind everything more urgent (FIFO per ring).
                w1t = w1p.tile([128, DC, H], BF16, tag="w1sb")
                nc.scalar.dma_start(
                    out=w1t[:], in_=w1_d[e].rearrange("(dc p) h -> p dc h", p=128)
                )

                def w1sel(dc, hc, w1t=w1t):
                    return w1t[:, dc, hc * 128:(hc + 1) * 128]

            # w2[e]: only needed once expert e's phase 2 starts.
            w2sb = w2p.tile([128, HC, D], BF16, tag="w2sb")
            w2_src = w2_d[e].rearrange("(hc p) d -> p hc d", p=128)
            if e == 0:
                half = HC // 2
                nc.sync.dma_start(out=w2sb[:, :half, :], in_=w2_src[:, :half, :])
                nc.scalar.dma_start(out=w2sb[:, half:, :], in_=w2_src[:, half:, :])
            else:
                nc.sync.dma_start(out=w2sb[:], in_=w2_src)

            # Phase 1: g[hc] = gelu(w1[e].T @ x.T + b1[e])   in [H, tok] layout
            g_tiles = []
            if e == 0:
                # First expert: its w1/x chunks are still streaming in, and a
                # (hc-outer, dc-inner) order would serialize on each arriving
                # (x[dc], w1[dc]) pair.  Run dc as the outer loop over 8
                # concurrently-open PSUM groups so every landed chunk unlocks
                # 8 matmuls.
                for hc in range(HC):
                    g = gp.tile([128, TOK], BF16, tag="g", name=f"g0_{hc}")
                    g_tiles.append(g)
                ph8 = []
                for hc in range(HC):
                    pool, tg = (php, "ph") if hc < 4 else (pop, "po")
                    p8 = pool.tile([128, T], F32, tag=tg, name=f"ph8_{hc}")
                    ph8.append(p8)
                for dc in range(DC):
                    for hc in range(HC):
                        nc.tensor.matmul(
                            ph8[hc][:],
                            lhsT=w1sel(dc, hc),
                            rhs=xsb[dc][0][:],
                            start=(dc == 0),
                            stop=(dc == DC - 1),
                        )
                for hc in range(HC):
                    nc.scalar.activation(
                        g_tiles[hc][:, 0:T], ph8[hc][:], GELU,
                        bias=b1_ap(hc, e),
                    )
                tt_range = range(1, NT)
            else:
                tt_range = range(NT)
            for hc in range(HC):
                if e == 0:
                    g = g_tiles[hc]
                else:
                    g = gp.tile([128, TOK], BF16, tag="g")
                    g_tiles.append(g)
                for tt in tt_range:
                    ph = php.tile([128, T], F32, tag="ph")
                    for dc in range(DC):
                        nc.tensor.matmul(
                            ph[:],
                            lhsT=w1sel(dc, hc),
                            rhs=xsb[dc][tt][:],
                            start=(dc == 0),
                            stop=(dc == DC - 1),
                        )
                    nc.scalar.activation(
                        g[:, tt * T:(tt + 1) * T],
                        ph[:],
                        GELU,
                        bias=b1_ap(hc, e),
                    )

            # Phase 2: acc[tc] += w[tok, e] * (g.T @ w2[e])   in [tok, D] layout
            for tcb in range(TC):
                last_group = e == E - 1 and tcb == TC - 1
                # The very last group is split into two N=256 half-groups so
                # its DVE drain + store overlap the final matmuls instead of
                # serializing into the kernel tail.
                halves = 2 if last_group else 1
                dw = D // halves
                for hf in range(halves):
                    d0 = hf * dw
                    po = pop.tile([128, dw], F32, tag="po", name=f"po_{hf}")
                    for hc in range(HC):
                        nc.tensor.matmul(
                            po[:],
                            lhsT=g_tiles[hc][:, tcb * 128:(tcb + 1) * 128],
                            rhs=w2sb[:, hc, d0:d0 + dw],
                            start=(hc == 0),
                            stop=(hc == HC - 1),
                        )
                    a = acc[tcb][:, d0:d0 + dw]
                    w_pp = w_pp_ap(tcb, e)
                    if e == 0:
                        nc.vector.tensor_scalar_mul(a, po[:], w_pp)
                    elif e < E - 1:
                        nc.vector.scalar_tensor_tensor(
                            a, po[:], w_pp, a, MULT, ADD
                        )
                    else:
                        # Final expert: the fused op writes the bf16 staging
                        # tile directly (half the store bytes; host upcasts).
                        ob = outp.tile([128, dw], BF16, tag="ob", name=f"ob_{hf}")
                        nc.vector.scalar_tensor_tensor(
                            ob[:], po[:], w_pp, a, MULT, ADD
                        )
                        nc.sync.dma_start(
                            out=out_d[tcb * 128:(tcb + 1) * 128, d0:d0 + dw],
                            in_=ob[:],
                        )

    nc.compile()
    return nc


def _get_nc():
    if "nc" not in _cache:
        _cache["nc"] = _build()
    return _cache["nc"]


def run(inputs: dict, trace: bool = False):
    x = np.ascontiguousarray(np.asarray(inputs["x"], dtype=np.float32))
    weights = np.ascontiguousarray(np.asarray(inputs["weights"], dtype=np.float32))
    w1 = np.ascontiguousarray(np.asarray(inputs["w1"], dtype=np.float32))
    b1 = np.ascontiguousarray(np.asarray(inputs["b1"], dtype=np.float32))
    w2 = np.ascontiguousarray(np.asarray(inputs["w2"], dtype=np.float32))
    b2 = np.ascontiguousarray(np.asarray(inputs["b2"], dtype=np.float32))

    x2 = x.reshape(B * L, D)
    wt2 = weights.reshape(B * L, E)
    b1p = np.ascontiguousarray(
        b1.T.reshape(HC, 128, E).transpose(1, 0, 2).reshape(128, HC * E)
    )
    w1h = np.ascontiguousarray(w1.astype(BF16_NP))
    w2h = np.ascontiguousarray(w2.astype(BF16_NP))

    in_maps = []
    for i in range(N_CORES):
        sl = slice(i * TOK, (i + 1) * TOK)
        wtp = (
            wt2[sl].reshape(TC, 128, E).transpose(1, 0, 2).reshape(128, TC * E)
        )
        cst = np.concatenate([wtp, b1p], axis=1)
        in_maps.append(
            {
                "xT": np.ascontiguousarray(x2[sl].T.astype(BF16_NP)),
                "cst": np.ascontiguousarray(cst),
                "w1": w1h,
                "w2": w2h,
            }
        )

    nc = _get_nc()
    res = run_bass_kernel_spmd(nc, in_maps, list(range(N_CORES)), trace=trace)
    _cache["last_res"] = res

    out = np.empty((B * L, D), dtype=np.float32)
    for i in range(N_CORES):
        out[i * TOK:(i + 1) * TOK] = res.results[i]["out"].astype(np.float32)

    # Rank-1 correction for b2: sum_e weights[t,e] * b2[e,:]
    out += wt2 @ b2
    return out.reshape(B, L, D), res.exec_time_ns


def kernel(**inputs) -> np.ndarray:
    out, _ = run(inputs)
    return out


# revision 39
# speedup vs baseline: 1.0063x; 1.0063x over previous
"""BatchedExpertPool Trainium2 kernel.

Computes, for x:[B,L,D], weights:[B,L,E], w1:[E,D,H], b1:[E,H],
w2:[E,H,D], b2:[E,D]:

    h   = gelu(einsum('bld,edh->bleh', x, w1) + b1)      (exact erf gelu)
    out = einsum('bleh,ehd->bled', h, w2) + b2
    ret = einsum('bled,ble->bld', out, weights)

Strategy: data-parallel over the B*L tokens across 8 NeuronCores, expert
params replicated.  Each core processes 1024 tokens entirely on-chip:

  - All matmul operands are bf16 (host-converted).  bf16 stationaries take
    the FWL LDWEIGHTS path (~119ns, fully hidden under the 216ns N=512
    stream); f32r's ~224ns weight load would bind the pipeline instead.
  - Phase 1 computes g = gelu(w1[e].T @ x^T + b1) in [H, tok] layout
    (lhsT = w1 chunk, moving = x^T), accumulating fp32 in PSUM.
  - Phase 2 is flipped to out[tok, D]: lhsT = g chunk (stationary),
    moving = w2[e] rows.  The per-token expert weight is then a
    per-PARTITION scalar, so the weighted accumulation is a single fused
    DVE op: acc = (psum * w[:,None]) + acc, and the token weights for all
    experts load in one 32KB DMA instead of 8 broadcast DMAs.
  - The b2 term (weights @ b2, a rank-1 correction) is added on the host.

Schedule notes (from trace iteration):
  - Prologue to first instruction is ~7us of framework cost (BSP start
    barrier + engine TENSOR_LOADs); scratch warm-up matmuls cover the HAM
    clock-gate ramp while the first x / w1[0] bytes land.
  - All loads ride the two physical HWDGE rings (sync, scalar) in strict
    criticality order; a third (SWDGE) ring would steal a 1/3 round-robin
    share of the ~358GB/s per-core HBM bandwidth from the critical bytes.
  - Token weights and b1 are packed into a single 64KB constant DMA.
  - The output is stored as bf16 (host upcasts) to halve the tail store,
    and the very last matmul group is split in half (N=256) so the final
    DVE drain + store overlap the last matmuls instead of serializing.

Steady state measured on hardware: median inter-matmul gap 0ns, every
matmul at the 216ns N=512 floor; the kernel is within ~4us of the
architectural minimum (221us streaming + fixed prologue/epilogue).
"""

import ml_dtypes
import numpy as np
from contextlib import ExitStack

import concourse.bass as bass
import concourse.tile as tile
from concourse import bacc, mybir
from concourse.bass_utils import run_bass_kernel_spmd

BF16_NP = ml_dtypes.bfloat16

B, L, D, E, H = 4, 2048, 512, 8, 1024
N_CORES = 8
TOK = (B * L) // N_CORES  # tokens per core
T = 512                   # matmul moving-dim tile (one PSUM bank of fp32)
NT = TOK // T             # token tiles per core (phase 1 moving dim)
TC = TOK // 128           # token chunks (phase 2 output partitions)
DC = D // 128             # D chunks
HC = H // 128             # H chunks
N_WARMUP_MM = 10

F32 = mybir.dt.float32
BF16 = mybir.dt.bfloat16
GELU = mybir.ActivationFunctionType.Gelu
MULT = mybir.AluOpType.mult
ADD = mybir.AluOpType.add

_cache: dict = {}


def _build():
    nc = bacc.Bacc(trn_type="TRN2", target_bir_lowering=False, debug=False)

    xT_d = nc.dram_tensor("xT", [D, TOK], BF16, kind="ExternalInput").ap()
    # cst packs the per-token expert weights [128, tc*E] and b1 [128, hc*E]
    # side by side: one DMA, one completion receipt.
    cst_d = nc.dram_tensor("cst", [128, (TC + HC) * E], F32, kind="ExternalInput").ap()
    w1_d = nc.dram_tensor("w1", [E, D, H], BF16, kind="ExternalInput").ap()
    w2_d = nc.dram_tensor("w2", [E, H, D], BF16, kind="ExternalInput").ap()
    out_d = nc.dram_tensor("out", [TOK, D], BF16, kind="ExternalOutput").ap()

    with tile.TileContext(nc) as tc, ExitStack() as ctx:
        consts = ctx.enter_context(tc.tile_pool(name="consts", bufs=1))
        w1p0 = ctx.enter_context(tc.tile_pool(name="w1p0", bufs=4))
        w1p = ctx.enter_context(tc.tile_pool(name="w1p", bufs=3))
        w2p = ctx.enter_context(tc.tile_pool(name="w2p", bufs=2))
        gp = ctx.enter_context(tc.tile_pool(name="gp", bufs=10))
        outp = ctx.enter_context(tc.tile_pool(name="outp", bufs=4))
        php = ctx.enter_context(tc.tile_pool(name="php", bufs=4, space="PSUM"))
        pop = ctx.enter_context(tc.tile_pool(name="pop", bufs=4, space="PSUM"))

        # PE warm-up on scratch data while the first tiles load: the HAM
        # clock gate needs ~3.4us of sustained activity to lift the PE from
        # 1.2 to 2.4 GHz.  memsets on the vector queue (gpsimd's initial
        # drain would delay them past the first DMA descriptors).
        wscr = consts.tile([128, 128], BF16)
        rscr = consts.tile([128, T], BF16)
        nc.vector.memset(wscr[:], 0.0)
        nc.vector.memset(rscr[:], 0.0)
        pscr = php.tile([128, T], F32, tag="ph")
        for _ in range(N_WARMUP_MM):
            nc.tensor.matmul(
                pscr[:], lhsT=wscr[:], rhs=rscr[:], start=True, stop=True,
            )

        # ALL loads go through the two physical HWDGE rings (sync, scalar) in
        # strict criticality order — a third (SWDGE) ring would steal a 1/3
        # round-robin share of the ~358GB/s per-core HBM bandwidth from the
        # first-expert-critical bytes.  x[dc] heads the sync ring while
        # w1[0][dc] heads the scalar ring: the dc-th matmul group of expert 0
        # needs exactly (x[dc], w1[0][dc]).  The packed constants follow the
        # x chunks (first needed by the gelu of expert 0, ~2us later).
        xsb = []
        for dc in range(DC):
            xt = consts.tile([128, TOK], BF16, name=f"x{dc}", tag=f"x{dc}")
            nc.sync.dma_start(out=xt[:], in_=xT_d[dc * 128:(dc + 1) * 128, :])
            xsb.append([xt[:, tt * T:(tt + 1) * T] for tt in range(NT)])

        cst = consts.tile([128, (TC + HC) * E], F32)
        nc.sync.dma_start(out=cst[:], in_=cst_d)

        def w_pp_ap(tcb, e):
            # per-partition token weight column for (token chunk, expert)
            return cst[:, tcb * E + e:tcb * E + e + 1]

        def b1_ap(hc, e):
            return cst[:, (TC + hc) * E + e:(TC + hc) * E + e + 1]

        # fp32 accumulators, one [128 tok, D] tile per token chunk
        acc = [
            consts.tile([128, D], F32, name=f"acc{t}", tag=f"acc{t}")
            for t in range(TC)
        ]

        for e in range(E):
            # w1[e]: expert 0 per-dc chunks (critical path: each landed chunk
            # unlocks matmuls); later experts in one DMA.
            if e == 0:
                w1t = []
                for dc in range(DC):
                    t = w1p0.tile([128, H], BF16, tag="w1e0")
                    nc.scalar.dma_start(
                        out=t[:], in_=w1_d[e, dc * 128:(dc + 1) * 128, :]
                    )
                    w1t.append(t)

                def w1sel(dc, hc, w1t=w1t):
                    return w1t[dc][:, hc * 128:(hc + 1) * 128]
            else:
                # Steady state: w1[e] on the scalar ring, w2[e] on the sync
                # ring, each behind everything more urgent (FIFO per ring).
                w1t = w1p.tile([128, DC, H], BF16, tag="w1sb")
                nc.scalar.dma_start(
                    out=w1t[:], in_=w1_d[e].rearrange("(dc p) h -> p dc h", p=128)
                )

                def w1sel(dc, hc, w1t=w1t):
                    return w1t[:, dc, hc * 128:(hc + 1) * 128]

            # w2[e]: only needed once expert e's phase 2 starts.
            w2sb = w2p.tile([128, HC, D], BF16, tag="w2sb")
            w2_src = w2_d[e].rearrange("(hc p) d -> p hc d", p=128)
            if e == 0:
                half = HC // 2
                nc.sync.dma_start(out=w2sb[:, :half, :], in_=w2_src[:, :half, :])
                nc.scalar.dma_start(out=w2sb[:, half:, :], in_=w2_src[:, half:, :])
            else:
                nc.sync.dma_start(out=w2sb[:], in_=w2_src)

            # Phase 1: g[hc] = gelu(w1[e].T @ x.T + b1[e])   in [H, tok] layout
            g_tiles = []
            if e == 0:
                # First expert: its w1/x chunks are still streaming in, and a
                # (hc-outer, dc-inner) order would serialize on each arriving
                # (x[dc], w1[dc]) pair.  Run dc as the outer loop over 8
                # concurrently-open PSUM groups so every landed chunk unlocks
                # 8 matmuls.
                for hc in range(HC):
                    g = gp.tile([128, TOK], BF16, tag="g", name=f"g0_{hc}")
                    g_tiles.append(g)
                ph8 = []
                for hc in range(HC):
                    pool, tg = (php, "ph") if hc < 4 else (pop, "po")
                    p8 = pool.tile([128, T], F32, tag=tg, name=f"ph8_{hc}")
                    ph8.append(p8)
                for dc in range(DC):
                    for hc in range(HC):
                        nc.tensor.matmul(
                            ph8[hc][:],
                            lhsT=w1sel(dc, hc),
                            rhs=xsb[dc][0][:],
                            start=(dc == 0),
                            stop=(dc == DC - 1),
                        )
                for hc in range(HC):
                    nc.scalar.activation(
                        g_tiles[hc][:, 0:T], ph8[hc][:], GELU,
                        bias=b1_ap(hc, e),
                    )
                tt_range = range(1, NT)
            else:
                tt_range = range(NT)
            for hc in range(HC):
                if e == 0:
                    g = g_tiles[hc]
                else:
                    g = gp.tile([128, TOK], BF16, tag="g")
                    g_tiles.append(g)
                for tt in tt_range:
                    ph = php.tile([128, T], F32, tag="ph")
                    for dc in range(DC):
                        nc.tensor.matmul(
                            ph[:],
                            lhsT=w1sel(dc, hc),
                            rhs=xsb[dc][tt][:],
                            start=(dc == 0),
                            stop=(dc == DC - 1),
                        )
                    nc.scalar.activation(
                        g[:, tt * T:(tt + 1) * T],
                        ph[:],
                        GELU,
                        bias=b1_ap(hc, e),
                    )

            # Phase 2: acc[tc] += w[tok, e] * (g.T @ w2[e])   in [tok, D] layout
            for tcb in range(TC):
                last_group = e == E - 1 and tcb == TC - 1
                # The very last group is split into two N=256 half-groups so
                # its DVE drain + store overlap the final matmuls instead of
                # serializing into the kernel tail.
                halves = 2 if last_group else 1
                dw = D // halves
                for hf in range(halves):
                    d0 = hf * dw
                    po = pop.tile([128, dw], F32, tag="po", name=f"po_{hf}")
                    for hc in range(HC):
                        nc.tensor.matmul(
                            po[:],
                            lhsT=g_tiles[hc][:, tcb * 128:(tcb + 1) * 128],
                            rhs=w2sb[:, hc, d0:d0 + dw],
                            start=(hc == 0),
                            stop=(hc == HC - 1),
                        )
                    a = acc[tcb][:, d0:d0 + dw]
                    w_pp = w_pp_ap(tcb, e)
                    if e == 0:
                        nc.vector.tensor_scalar_mul(a, po[:], w_pp)
                    elif e < E - 1:
                        nc.vector.scalar_tensor_tensor(
                            a, po[:], w_pp, a, MULT, ADD
                        )
                    else:
                        # Final expert: the fused op writes the bf16 staging
                        # tile directly (half the store bytes; host upcasts).
                        ob = outp.tile([128, dw], BF16, tag="ob", name=f"ob_{hf}")
                        nc.vector.scalar_tensor_tensor(
                            ob[:], po[:], w_pp, a, MULT, ADD
                        )
                        nc.sync.dma_start(
                            out=out_d[tcb * 128:(tcb + 1) * 128, d0:d0 + dw],
                            in_=ob[:],
                        )

    nc.compile()
    return nc


def _get_nc():
    if "nc" not in _cache:
        _cache["nc"] = _build()
    return _cache["nc"]


def run(inputs: dict, trace: bool = False):
    x = np.ascontiguousarray(np.asarray(inputs["x"], dtype=np.float32))
    weights = np.ascontiguousarray(np.asarray(inputs["weights"], dtype=np.float32))
    w1 = np.ascontiguousarray(np.asarray(inputs["w1"], dtype=np.float32))
    b1 = np.ascontiguousarray(np.asarray(inputs["b1"], dtype=np.float32))
    w2 = np.ascontiguousarray(np.asarray(inputs["w2"], dtype=np.float32))
    b2 = np.ascontiguousarray(np.asarray(inputs["b2"], dtype=np.float32))

    x2 = x.reshape(B * L, D)
    wt2 = weights.reshape(B * L, E)
    b1p = np.ascontiguousarray(
        b1.T.reshape(HC, 128, E).transpose(1, 0, 2).reshape(128, HC * E)
    )
    w1h = np.ascontiguousarray(w1.astype(BF16_NP))
    w2h = np.ascontiguousarray(w2.astype(BF16_NP))

    in_maps = []
    for i in range(N_CORES):
        sl = slice(i * TOK, (i + 1) * TOK)
        wtp = (
            wt2[sl].reshape(TC, 128, E).transpose(1, 0, 2).reshape(128, TC * E)
        )
        cst = np.concatenate([wtp, b1p], axis=1)
        in_maps.append(
            {
                "xT": np.ascontiguousarray(x2[sl].T.astype(BF16_NP)),
                "cst": np.ascontiguousarray(cst),
                "w1": w1h,
                "w2": w2h,
            }
        )

    nc = _get_nc()
    res = run_bass_kernel_spmd(nc, in_maps, list(range(N_CORES)), trace=trace)
    _cache["last_res"] = res

    out = np.empty((B * L, D), dtype=np.float32)
    for i in range(N_CORES):
        out[i * TOK:(i + 1) * TOK] = res.results[i]["out"].astype(np.float32)

    # Rank-1 correction for b2: sum_e weights[t,e] * b2[e,:]
    out += wt2 @ b2
    return out.reshape(B, L, D), res.exec_time_ns


def kernel(**inputs) -> np.ndarray:
    out, _ = run(inputs)
    return out
